# revision 10
# baseline (speedup 1.0000x reference)
"""CosFormer layer kernel v2 for 8x Trainium2 (Bass/Tile), data-parallel over batch.

Restructured from baseline (454us) based on trace evidence:
- PE issues N=512 MMs at 216ns warm/unstalled (LDWEIGHTS fully hidden by the
  64-deep reorder window) -> the win is PE density, not LDW dedup.
- Baseline lost ~226us to HAM re-throttle (K=4/8) during attention stalls and
  ~72us to ACT table swaps (Exp<->Reciprocal<->Sqrt interleaving).
Changes:
- Whole layer software-pipelined at c-block granularity: batch b's attention
  interleaves with batch b+1's QKV proj/norms, batch b-1's Oproj/LN1 tail, and
  per-token-block FFN chunks -> dense PE stream end to end.
- ACT runs only exp/relu/copy (one table) during the main stream; norms use raw
  Rsqrt batched per batch; softmax denominator reciprocal moved to DVE
  (reciprocal_approx_fast); LN2 Rsqrt fuses +eps via the bias operand.
- h1 -> h1T transposes moved from PE (64 transpose MMs + 64 ACT copies) to DMA
  xbar transposes ([128,128] bf16 blocks) on otherwise-idle DMA queues.
"""

import sys

if "/opt/trn_rl_repo" not in sys.path:
    sys.path.insert(0, "/opt/trn_rl_repo")

import ml_dtypes
import numpy as np

import concourse.bass as bass
import concourse.tile as tile
from concourse import mybir
from concourse.bass_utils import run_bass_kernel_spmd

F32 = mybir.dt.float32
BF16 = mybir.dt.bfloat16
NPBF16 = ml_dtypes.bfloat16
AX = mybir.AxisListType
AF = mybir.ActivationFunctionType
OP = mybir.AluOpType

# problem constants
B, S, D = 32, 512, 512
H, DK, DV, DFF = 8, 64, 64, 2048
TEMP = float(np.sqrt(DK))
LN_EPS = 1e-5
NCORES = 8
BPC = B // NCORES          # batches per core
T = BPC * S                # tokens per core
DC = D // 128              # d chunks
FC = DFF // 128            # dff chunks
SB = S // 128              # token chunks per batch
P = 128

# h1 -> h1T via DMA xbar transpose (frees 64 PE transposes + 64 ACT copies).
# Flip to False to fall back to PE transposes if the SBUF->SBUF path misbehaves.
USE_DMA_TRANSPOSE = True


def ts(i, n):
    return slice(i * n, (i + 1) * n)


# walrus codegen caps on semaphore-wait commands per instruction (empirical);
# excess waits are moved onto chained same-engine NOPs ahead of the instruction.
_WAIT_CAPS = {}
_DEFAULT_WAIT_CAP = 1
_NOP_WAIT_CAP = 1


def _legalize_waits(nc):
    nop_id = [0]
    for f in nc.m.functions:
        for bb in f.blocks:
            insts = bb.instructions
            i = 0
            while i < len(insts):
                ins = insts[i]
                si = ins.sync_info
                cap = _WAIT_CAPS.get(type(ins).__name__, _DEFAULT_WAIT_CAP)
                if si is not None and si.on_wait and len(si.on_wait) > cap:
                    waits = list(si.on_wait)
                    keep = waits[-cap:] if cap > 0 else []
                    excess = waits[: len(waits) - cap]
                    new_nops = []
                    for j in range(0, len(excess), _NOP_WAIT_CAP):
                        chunk = excess[j: j + _NOP_WAIT_CAP]
                        nop = mybir.InstNoOp(
                            name=f"waitnop-{nop_id[0]}",
                            engine=ins.engine,
                            ins=[],
                            outs=[],
                            sync_info=mybir.SyncInfo(on_wait=chunk, on_update=[]),
                        )
                        nop_id[0] += 1
                        nc.register_instruction(nop)
                        new_nops.append(nop)
                    si.on_wait[:] = keep
                    insts[i:i] = new_nops
                    i += len(new_nops)
                i += 1


def _dedup_ldweights(nc):
    """Remove an InstLdweights when the PE array already holds the same
    weights (sync-safe; waits migrate onto the next kept PE instruction)."""
    for f in nc.m.functions:
        for bb in f.blocks:
            insts = bb.instructions
            out = []
            last_w = None
            pend_waits = []
            for ins in insts:
                tn = type(ins).__name__
                if tn == "InstLdweights":
                    w = str(ins.ins[0])
                    if w == last_w:
                        si = ins.sync_info
                        if si is not None and si.on_wait:
                            pend_waits.extend(list(si.on_wait))
                        continue
                    last_w = w
                elif tn == "InstMatmult":
                    if getattr(ins, "is_transpose", False):
                        last_w = None
                if pend_waits and tn in ("InstLdweights", "InstMatmult"):
                    si = ins.sync_info
                    if si is None:
                        ins.sync_info = mybir.SyncInfo(on_wait=list(pend_waits),
                                                       on_update=[])
                    else:
                        si.on_wait.extend(pend_waits)
                    pend_waits = []
                out.append(ins)
            assert not pend_waits
            insts[:] = out


def _act_raw(nc, out, in_, func, scale=1.0, bias_ap=None):
    """Raw ACT-engine activation bypassing bass's accuracy guard (used for
    Reciprocal / Rsqrt; measured err ~1e-5 rel - far below bf16 noise)."""
    eng = nc.scalar
    inputs = [eng.lower_ap(in_)]
    if bias_ap is not None:
        inputs.append(eng.lower_ap(bias_ap))
    else:
        inputs.append(mybir.ImmediateValue(dtype=mybir.dt.float32, value=0.0))
    inputs.append(mybir.ImmediateValue(dtype=mybir.dt.float32, value=scale))
    inputs.append(mybir.ImmediateValue(dtype=mybir.dt.float32, value=0.0))
    return eng.add_instruction(
        mybir.InstActivation(
            name=nc.get_next_instruction_name(),
            func=func,
            ins=inputs,
            outs=[eng.lower_ap(out)],
        )
    )


def _act_rsqrt(nc, out, in_, scale=1.0, bias_ap=None):
    """out = 1/sqrt(in_*scale + bias). bias is an AP (walrus wants table-based
    activations' bias as an AP); defaults to the const-0 AP."""
    if bias_ap is None:
        bias_ap = nc.const_aps.scalar_like(0.0, in_)
    return _act_raw(nc, out, in_, AF.Rsqrt, scale=scale, bias_ap=bias_ap)


def _fast_recip(nc, pool, x_ap, shape, out_dtype, tagbase):
    """y = 1/x via ACT Reciprocal (correctness-only paths)."""
    y = pool.tile(shape, out_dtype, tag=tagbase + "_y", name=tagbase + "_y")
    _act_raw(nc, y[:], x_ap, AF.Reciprocal)
    return y


def build_program(apply_gb1=True, apply_gb2=True, apply_bf2=True, apply_bf1=True):
    # eps guard: with var ~1 the eps=1e-5 shift perturbs rstd by ~5e-6 relative
    # (below bf16 noise) and LN2 renormalizes exactly; LN1 fast path (g1=1,b1=0,
    # b_ff1=0) only needs the mean-subtract.
    ln1_fast = (not apply_gb1) and (not apply_bf1)
    nc = bass.Bass("TRN2", target_bir_lowering=False, debug=False)

    # ---- DRAM I/O ----
    x_d = nc.dram_tensor("x", [T, D], F32, kind="ExternalInput")
    xb_d = nc.dram_tensor("xb", [T, D], BF16, kind="ExternalInput")
    wq_d = nc.dram_tensor("wq", [D, D], BF16, kind="ExternalInput")
    wk_d = nc.dram_tensor("wk", [D, D], BF16, kind="ExternalInput")
    wv_d = nc.dram_tensor("wv", [D, D], BF16, kind="ExternalInput")
    wo_d = nc.dram_tensor("wo", [D, D], BF16, kind="ExternalInput")
    wf1_d = nc.dram_tensor("wf1", [D, DFF], BF16, kind="ExternalInput")
    wf2_d = nc.dram_tensor("wf2", [DFF, D], BF16, kind="ExternalInput")
    bf1_d = nc.dram_tensor("bf1", [P, FC], F32, kind="ExternalInput")
    bf2_d = nc.dram_tensor("bf2", [1, D], BF16, kind="ExternalInput")
    g1_d = nc.dram_tensor("g1b", [P, D], F32, kind="ExternalInput")
    b1_d = nc.dram_tensor("b1b", [P, D], F32, kind="ExternalInput")
    g2_d = nc.dram_tensor("g2b", [P, D], F32, kind="ExternalInput")
    b2_d = nc.dram_tensor("b2b", [P, D], F32, kind="ExternalInput")
    id_d = nc.dram_tensor("ident", [P, P], F32, kind="ExternalInput")
    ssum_d = nc.dram_tensor("selsum", [P, DC, H], BF16, kind="ExternalInput")
    sbc_d = nc.dram_tensor("selbc", [H, DC, P], BF16, kind="ExternalInput")
    ones_d = nc.dram_tensor("ones_row", [1, P], BF16, kind="ExternalInput")
    out_d = nc.dram_tensor("out", [T, D], F32, kind="ExternalOutput")
    h1scr_d = nc.dram_tensor("h1scr", [T, D], BF16, kind="Internal")

    with tile.TileContext(nc) as tc:
        with tc.tile_pool(name="consts", bufs=1) as consts, \
             tc.tile_pool(name="pers", bufs=1) as pers, \
             tc.tile_pool(name="work", bufs=1) as wk_, \
             tc.tile_pool(name="batch", bufs=2) as bp, \
             tc.tile_pool(name="epool", bufs=3) as ep, \
             tc.tile_pool(name="tmp", bufs=2) as bt, \
             tc.tile_pool(name="psSC", bufs=2, space="PSUM") as psSC, \
             tc.tile_pool(name="psAV", bufs=1, space="PSUM") as psAV, \
             tc.tile_pool(name="psM", bufs=2, space="PSUM") as psM:

            # ---- persistent tensors + startup DMAs ----
            # Both HWDGE queues serialize descriptor generation (~0.7us per
            # DMA, ~1.3us per xbar transpose), so startup order = need order:
            # wq/wk + the hseg-0 xT transposes gate proj(0). Split the eight
            # xT transposes across sync and ACT queues; consts and the 8MB of
            # FFN weights come after.
            h1T = pers.tile([P, DC, T], BF16)
            # bf16 h1 (residual source + DMA-transpose source): the ~4e-3
            # rounding is renormalized by LN2; saves 16KB/partition SBUF.
            h1tok = pers.tile([P, T // P, D], BF16)
            xT = wk_.tile([P, DC, T], BF16)
            wq = pers.tile([P, DC, D], BF16)
            nc.sync.dma_start(wq[:], wq_d.ap().rearrange("(c p) n -> p c n", p=P))
            wk = pers.tile([P, DC, D], BF16)
            nc.sync.dma_start(wk[:], wk_d.ap().rearrange("(c p) n -> p c n", p=P))

            def xt_chunk(b):
                # per-batch [S,128]->[128,S] transposes, split across both
                # HWDGE queues so batch 0's chunks land first (~8us) and
                # proj(0) starts ~15us earlier than with half-T transposes
                for eng, clist in ((nc.sync, (0, 1)), (nc.scalar, (2, 3))):
                    for c in clist:
                        eng.dma_start_transpose(xT[:, c, ts(b, S)],
                                                xb_d[ts(b, S), ts(c, P)])
            xt_chunk(0)

            # ---- constants (ACT queue: sync carries the weight loads) ----
            ident = consts.tile([P, P], F32)
            nc.scalar.dma_start(ident[:], id_d[:])
            identb = None
            if not USE_DMA_TRANSPOSE:
                identb = consts.tile([P, P], BF16)
                nc.vector.tensor_copy(identb[:], ident[:])
            selsum = consts.tile([P, DC, H], BF16)
            nc.scalar.dma_start(selsum[:], ssum_d[:])
            selbc = consts.tile([H, DC, P], BF16)
            nc.scalar.dma_start(selbc[:], sbc_d[:])
            ones_row = consts.tile([1, P], BF16)
            nc.scalar.dma_start(ones_row[:], ones_d[:])
            g1b = b1b = g2b = b2b = bf1 = bf2 = None
            if apply_gb1:
                g1b = consts.tile([P, D], F32)
                nc.scalar.dma_start(g1b[:], g1_d[:])
                b1b = consts.tile([P, D], F32)
                nc.scalar.dma_start(b1b[:], b1_d[:])
            if apply_gb2:
                g2b = consts.tile([P, D], F32)
                nc.scalar.dma_start(g2b[:], g2_d[:])
                b2b = consts.tile([P, D], F32)
                nc.scalar.dma_start(b2b[:], b2_d[:])
            if apply_bf1:
                bf1 = consts.tile([P, FC], F32)
                nc.scalar.dma_start(bf1[:], bf1_d[:])
            if apply_bf2:
                bf2 = consts.tile([1, D], BF16)
                nc.scalar.dma_start(bf2[:], bf2_d[:])
            eps128 = consts.tile([P, 1], F32)
            nc.vector.memset(eps128[:], LN_EPS)

            wv = pers.tile([P, DC, D], BF16)
            nc.sync.dma_start(wv[:], wv_d.ap().rearrange("(c p) n -> p c n", p=P))
            wo = pers.tile([P, DC, D], BF16)
            nc.sync.dma_start(wo[:], wo_d.ap().rearrange("(c p) n -> p c n", p=P))
            xt_chunk(1)
            xt_chunk(2)
            xt_chunk(3)
            wf1 = pers.tile([P, DC, DFF], BF16)
            nc.sync.dma_start(wf1[:], wf1_d.ap().rearrange("(c p) n -> p c n", p=P))
            wf2 = pers.tile([P, FC, D], BF16)
            nc.sync.dma_start(wf2[:], wf2_d.ap().rearrange("(c p) n -> p c n", p=P))

            # per-batch state (cycled, bufs=2)
            QTs, KTs, Vbs, attbs, den8s, rden8s = {}, {}, {}, {}, {}, {}
            r8s = {}
            ffas = {}
            ln2st = {}

            # ---------------- emission helpers ----------------

            def run_gen(g):
                for _ in g:
                    pass

            def weave(a, b):
                """Alternate one chunk from each stream until both end."""
                a, b = iter(a), iter(b)
                alive = True
                while alive:
                    alive = False
                    for it in (a, b):
                        try:
                            next(it)
                            alive = True
                            yield
                        except StopIteration:
                            pass

            def chain(*gens):
                for g in gens:
                    for _ in g:
                        yield

            def proj_qk(b):
                """Q/K projections for batch b (feature-major, bf16).
                bufs=3: QT(b+2) is written (proj filler in S(b)) while
                scores(b, c>=2) still read QT(b)."""
                bcols = ts(b, S)
                QT = bp.tile([P, DC, S], BF16, tag="QT", name="QT", bufs=3)
                KT = bp.tile([P, DC, S], BF16, tag="KT", name="KT", bufs=3)
                QTs[b], KTs[b] = QT, KT
                for w_sb, XTt in ((wq, QT), (wk, KT)):
                    for cpair in range(DC // 2):
                        pp = psSC.tile([P, 2, S], F32, tag="pscr", name="pp")
                        for ci in range(2):
                            c = 2 * cpair + ci
                            for kc in range(DC):
                                nc.tensor.matmul(pp[:, ci, :],
                                                 w_sb[:, kc, ts(c, P)],
                                                 xT[:, kc, bcols],
                                                 start=(kc == 0),
                                                 stop=(kc == DC - 1))
                        for ci in range(2):
                            c = 2 * cpair + ci
                            nc.vector.tensor_copy(XTt[:, c, :], pp[:, ci, :])
                        yield

            def proj_v(b):
                """V projection for batch b (token-major, into augmented Vb).
                bufs=3: proj_v(b+2) may be pulled while AV(b) still reads
                Vb(b)."""
                Vb = bp.tile([P, SB, H, DV + 1], BF16, tag="Vb", name="Vb",
                             bufs=3)
                Vbs[b] = Vb
                nc.gpsimd.memset(Vb[:, :, :, DV:DV + 1], 1.0)
                for qpair in range(SB // 2):
                    pv = psSC.tile([P, 2, S], F32, tag="pscr", name="pv")
                    for i in range(2):
                        q = 2 * qpair + i
                        for kc in range(DC):
                            nc.tensor.matmul(pv[:, i, :],
                                             xT[:, kc, ts(b * SB + q, P)],
                                             wv[:, kc, :],
                                             start=(kc == 0), stop=(kc == DC - 1))
                    for i in range(2):
                        q = 2 * qpair + i
                        nc.vector.tensor_copy(
                            Vb[:, q, :, 0:DV],
                            pv[:, i, :].rearrange("p (h d) -> p h d", h=H))
                    yield

            def norms(b):
                """rq=1/(temp*||q||), rk=1/||k|| via Square + selector-matmul
                column sums + one raw Rsqrt per side."""
                for w2, (XTt, scale) in enumerate(
                        ((QTs[b], TEMP * TEMP), (KTs[b], 1.0))):
                    ps8 = psM.tile([H, S], F32, tag="misc", name="ps8")
                    for c in range(DC):
                        sq = bt.tile([P, S], BF16, tag="sq", name="sq")
                        # Square on ACT (same table as Exp) to offload DVE
                        nc.scalar.activation(sq[:], XTt[:, c, :], AF.Square)
                        nc.tensor.matmul(ps8[:], selsum[:, c, :], sq[:],
                                         start=(c == 0), stop=(c == DC - 1))
                    r8 = bt.tile([H, S], BF16, tag=f"r8_{w2}", name="r8",
                                 bufs=1)
                    _act_rsqrt(nc, r8[:], ps8[:], scale=scale)
                    r8s[(b, w2)] = r8
                    yield

            def folds(b, half):
                """Fold rq/rk into QT/KT via matmul-broadcast [8,S]->[128,S].
                half 0 -> c 0..1, half 1 -> c 2..3 (split for interleaving)."""
                for c in range(2 * half, 2 * half + 2):
                    for w2, XTt in enumerate((QTs[b], KTs[b])):
                        pb = psM.tile([P, S], F32, tag="misc", name="pb")
                        nc.tensor.matmul(pb[:], selbc[:, c, :], r8s[(b, w2)][:],
                                         start=True, stop=True)
                        nc.vector.tensor_mul(XTt[:, c, :], XTt[:, c, :], pb[:])
                    yield

            def attn_alloc(b):
                attbs[b] = bp.tile([P, DC, S], BF16, tag="attb", name="attb")
                den8s[b] = bt.tile([H, S], F32, tag="den8", name="den8")

            def attn_part1(b, c):
                """scores + exp + AV accumulation for head pair c of batch b.
                Scores run one j ahead of the AVs so the exp latency is
                covered; a filler chunk is pulled at every yield. Evacuates
                attb UNSCALED and DMA-packs the denominator rows into den8."""
                QT, KT, Vb = QTs[b], KTs[b], Vbs[b]
                attb = attbs[b]
                pav0 = psAV.tile([DV + 1, S], F32, tag="pav0", name="pav0")
                pav1 = psAV.tile([DV + 1, S], F32, tag="pav1", name="pav1")

                def scores(j):
                    pscr = psSC.tile([P, 2, S], F32, tag="pscr", name="pscr")
                    for h in range(2):
                        r0 = 64 * h
                        nc.tensor.matmul(pscr[:, h, :],
                                         KT[r0:r0 + 64, c, ts(j, P)],
                                         QT[r0:r0 + 64, c, :],
                                         start=True, stop=True)
                    e = ep.tile([P, 2, S], BF16, tag="e", name="e")
                    nc.scalar.activation(e[:], pscr[:], AF.Exp)
                    return e

                es = [scores(0)]
                for j in range(SB):
                    if j + 1 < SB:
                        es.append(scores(j + 1))
                    yield
                    for h, pav in ((0, pav0), (1, pav1)):
                        nc.tensor.matmul(pav[:], Vb[:, j, 2 * c + h, :],
                                         es[j][:, h, :],
                                         start=(j == 0), stop=(j == SB - 1))
                    yield
                for h, pav in ((0, pav0), (1, pav1)):
                    nc.vector.tensor_copy(attb[64 * h:64 * h + 64, c, :],
                                          pav[0:DV, :])
                    # den row evac on ACT (Copy shares the Exp table) then
                    # DMA SBUF->SBUF to pack partition row 2c+h of den8
                    den_sb = bt.tile([1, S], F32, tag="densb", name="densb")
                    nc.scalar.copy(den_sb[:], pav[DV:DV + 1, :])
                    r = 2 * c + h
                    nc.sync.dma_start(den8s[b][r:r + 1, :], den_sb[:])
                yield

            def attn_finish_a(b):
                """One [8,S] reciprocal for the batch (DVE only)."""
                rden8 = bt.tile([H, S], F32, tag="rden8", name="rden8", bufs=1)
                nc.vector.reciprocal(rden8[:], den8s[b][:])
                rden8b = bt.tile([H, S], BF16, tag="rden8b", name="rden8b",
                                 bufs=1)
                nc.vector.tensor_copy(rden8b[:], rden8[:])
                rden8s[b] = rden8b
                yield

            def attn_finish_b(b):
                """Per-c broadcast (selbc matmul) + in-place scale of attb."""
                attb = attbs[b]
                rden8b = rden8s.pop(b)
                for c in range(DC):
                    pb = psM.tile([P, S], F32, tag="misc", name="pbden")
                    nc.tensor.matmul(pb[:], selbc[:, c, :], rden8b[:],
                                     start=True, stop=True)
                    nc.vector.tensor_mul(attb[:, c, :], attb[:, c, :], pb[:])
                    yield

            def attn_finish(b):
                for _ in attn_finish_a(b):
                    yield
                for _ in attn_finish_b(b):
                    yield

            def tail_xt2(b):
                """Prefetch this batch's residual x rows (token-major, f32)."""
                tiles = []
                for q in range(SB):
                    xt2 = bt.tile([P, D], F32, tag=f"xt2_{q}", name="xt2",
                                  bufs=1)
                    nc.sync.dma_start(xt2[:], x_d[ts(b * SB + q, P), :])
                    tiles.append(xt2)
                return tiles

            def tail_q(b, q, xt2):
                """O-projection + residual + LN1 + h1 scratch store."""
                attb = attbs[b]
                po = psM.tile([P, D], F32, tag="misc", name="po")
                for c in range(DC):
                    nc.tensor.matmul(po[:], attb[:, c, ts(q, P)], wo[:, c, :],
                                     start=(c == 0), stop=(c == DC - 1))
                idx = b * SB + q
                h1 = h1tok[:, idx, :]
                if ln1_fast:
                    r1 = bt.tile([P, D], F32, tag="r1", name="r1", bufs=1)
                    s1 = bt.tile([P, 1], F32, tag="s1", name="s1")
                    nc.vector.scalar_tensor_tensor(
                        r1[:], po[:], 1.0, xt2[:],
                        op0=OP.mult, op1=OP.add, accum_out=s1[:])
                    nm = bt.tile([P, 1], F32, tag="nm", name="nm")
                    nc.vector.tensor_scalar_mul(nm[:], s1[:], -1.0 / D)
                    nc.vector.tensor_scalar_add(h1, r1[:], nm[:])
                else:
                    r1 = bt.tile([P, D], F32, tag="r1", name="r1", bufs=1)
                    nc.vector.tensor_add(r1[:], po[:], xt2[:])
                    bst = bt.tile([P, 6], F32, tag="bst", name="bst")
                    nc.vector.bn_stats(bst[:], r1[:])
                    mv = bt.tile([P, 2], F32, tag="mv", name="mv")
                    nc.vector.bn_aggr(mv[:], bst[:])
                    rstd = bt.tile([P, 1], F32, tag="rstd", name="rstd")
                    _act_rsqrt(nc, rstd[:], mv[:, 1:2], bias_ap=eps128[:])
                    nc.vector.tensor_scalar(h1, r1[:], mv[:, 0:1], rstd[:],
                                            OP.subtract, OP.mult)
                    if apply_gb1:
                        nc.vector.tensor_mul(h1, h1, g1b[:])
                        nc.vector.tensor_add(h1, h1, b1b[:])
                if USE_DMA_TRANSPOSE:
                    # store token-major h1 to scratch DRAM; load_h1T() later
                    # transpose-loads a whole batch in 4 big xbar transposes
                    nc.sync.dma_start(h1scr_d[ts(idx, P), :], h1tok[:, idx, :])
                else:
                    for c in range(DC):
                        pt2 = psM.tile([P, P], BF16, tag="miscT", name="pt2")
                        nc.tensor.transpose(pt2[:], h1tok[:, idx, ts(c, P)],
                                            identb[:])
                        nc.vector.tensor_copy(h1T[:, c, ts(idx, P)], pt2[:])
                yield

            def load_h1T(b):
                """Transpose-load batch b's h1 (DRAM scratch, token-major)
                into feature-major h1T: 4 [S,128]->[128,S] xbar transposes."""
                if not USE_DMA_TRANSPOSE:
                    return
                for c in range(DC):
                    nc.sync.dma_start_transpose(
                        h1T[:, c, ts(b, S)], h1scr_d[ts(b, S), ts(c, P)])

            def ffn1_chunk(tb, f_lo, f_hi):
                """FFN1 (relu(W1^T h1^T)) for f chunks [f_lo, f_hi).
                ffa bufs=1: the schedule finishes FFN2(tb) before FFN1(tb+1)
                starts, so one slot cycles."""
                if tb not in ffas:
                    ffas[tb] = bp.tile([P, FC, S], BF16, tag="ffa",
                                       name="ffa", bufs=1)
                ffa = ffas[tb]
                for f in range(f_lo, f_hi):
                    pf = psM.tile([P, S], F32, tag="misc", name="pf")
                    for c in range(DC):
                        nc.tensor.matmul(pf[:], wf1[:, c, ts(f, P)],
                                         h1T[:, c, ts(tb, S)],
                                         start=(c == 0), stop=(c == DC - 1))
                    relu_bias = bf1[:, f:f + 1] if apply_bf1 else 0.0
                    nc.scalar.activation(ffa[:, f, :], pf[:],
                                         AF.Relu, bias=relu_bias)
                    yield

            def ffn2_q(tb, q):
                """FFN2 + residual + LN2 stats for token block q of tb."""
                if tb not in ln2st:
                    var4 = bt.tile([P, SB], F32, tag="var4", name="var4")
                    mean4 = bt.tile([P, SB], F32, tag="mean4", name="mean4")
                    ln2st[tb] = (var4, mean4, [])
                var4, mean4, r2s = ln2st[tb]
                ffa = ffas[tb]
                p2 = psM.tile([P, D], F32, tag="misc", name="p2")
                for f in range(FC):
                    nc.tensor.matmul(p2[:], ffa[:, f, ts(q, P)],
                                     wf2[:, f, :], start=(f == 0),
                                     stop=(not apply_bf2 and f == FC - 1))
                    if f % 4 == 3 and f != FC - 1:
                        yield
                if apply_bf2:
                    nc.tensor.matmul(p2[:], ones_row[:], bf2[:],
                                     start=False, stop=True)
                r2 = bt.tile([P, D], F32, tag=f"r2_{q}", name="r2", bufs=1)
                nc.vector.tensor_add(r2[:], p2[:], h1tok[:, tb * SB + q, :])
                bst2 = bt.tile([P, 6], F32, tag="bst2", name="bst2")
                nc.vector.bn_stats(bst2[:], r2[:])
                mv2 = bt.tile([P, 2], F32, tag="mv2", name="mv2")
                nc.vector.bn_aggr(mv2[:], bst2[:])
                nc.vector.tensor_copy(var4[:, q:q + 1], mv2[:, 1:2])
                nc.vector.tensor_copy(mean4[:, q:q + 1], mv2[:, 0:1])
                r2s.append(r2)
                yield

            def ln2_finish(tb):
                var4, mean4, r2s = ln2st.pop(tb)
                rstd4 = bt.tile([P, SB], F32, tag="rstd4", name="rstd4")
                _act_rsqrt(nc, rstd4[:], var4[:], bias_ap=eps128[:])
                for q in range(SB):
                    # normalize in place (r2 tiles are per-q): the 4 q-chains
                    # stay independent so the out-DMAs pipeline
                    y = r2s[q]
                    nc.vector.tensor_scalar(y[:], y[:], mean4[:, q:q + 1],
                                            rstd4[:, q:q + 1],
                                            OP.subtract, OP.mult)
                    if apply_gb2:
                        nc.vector.tensor_mul(y[:], y[:], g2b[:])
                        nc.vector.tensor_add(y[:], y[:], b2b[:])
                    nc.sync.dma_start(out_d[ts(tb * SB + q, P), :], y[:])

            # ---------------- pipelined schedule ----------------
            # pre: batches 0,1 projected+normed, batch 0 folded
            for g in (proj_qk(0), proj_v(0), norms(0), proj_qk(1), proj_v(1),
                      folds(0, 0), folds(0, 1), norms(1)):
                run_gen(g)

            def seg(b, stream):
                """A(b) c-blocks; one filler chunk pulled per part1 yield,
                remainder drained at segment end."""
                attn_alloc(b)
                it = iter(stream)
                for c in range(DC):
                    for _ in attn_part1(b, c):
                        next(it, None)
                for _ in it:
                    pass

            def tails(b, xt2s):
                return chain(*(tail_q(b, q, xt2s[q]) for q in range(SB)))

            # S0: A(0) + folds(1) + proj(2) + norms(2)
            # norms(b+2) last: its ACT Rsqrt (different table) lands after
            # this segment's Exps.
            seg(0, chain(folds(1, 0), folds(1, 1), proj_qk(2), proj_v(2),
                         norms(2)))

            xt2s0 = tail_xt2(0)
            # S1: A(1) + finish(0)/T(0) + folds(2) + proj(3) + norms(3)
            seg(1, chain(attn_finish_a(0), folds(2, 0),
                         attn_finish_b(0),
                         weave(tails(0, xt2s0),
                               chain(proj_qk(3), folds(2, 1), proj_v(3))),
                         norms(3)))
            load_h1T(0)
            xt2s1 = tail_xt2(1)

            # S2: A(2) + finish(1)/T(1) + folds(3) + FFN1(0)
            # (ffa bufs=1 discipline: FFN2(tb) fully precedes FFN1(tb+1))
            seg(2, chain(attn_finish_a(1), folds(3, 0),
                         attn_finish_b(1),
                         weave(tails(1, xt2s1),
                               chain(folds(3, 1), ffn1_chunk(0, 0, 16)))))
            load_h1T(1)
            xt2s2 = tail_xt2(2)

            # S3: A(3) + finish(2)/T(2) + FFN2(0) then FFN1(1)
            seg(3, chain(attn_finish_a(2), ffn2_q(0, 0),
                         attn_finish_b(2),
                         weave(tails(2, xt2s2),
                               chain(ffn2_q(0, 1), ffn2_q(0, 2),
                                     ffn2_q(0, 3), ffn1_chunk(1, 0, 16)))))
            load_h1T(2)
            xt2s3 = tail_xt2(3)

            # S4: finish(3) + T(3) + FFN2(1), then FFN1(2)
            run_gen(attn_finish(3))
            ln2_finish(0)
            for q in range(SB):
                run_gen(tail_q(3, q, xt2s3[q]))
                run_gen(ffn2_q(1, q))
            load_h1T(3)
            run_gen(ffn1_chunk(2, 0, 8))
            ln2_finish(1)
            run_gen(ffn1_chunk(2, 8, 16))
            # S5: FFN2(2), then FFN1(3)
            for q in range(SB):
                run_gen(ffn2_q(2, q))
            run_gen(ffn1_chunk(3, 0, 8))
            ln2_finish(2)
            run_gen(ffn1_chunk(3, 8, 16))
            # S6: FFN2(3)
            for q in range(SB):
                run_gen(ffn2_q(3, q))
            ln2_finish(3)

    _dedup_ldweights(nc)
    _legalize_waits(nc)
    return nc


_CACHED_NC = {}


def _get_nc(flags):
    if flags not in _CACHED_NC:
        _CACHED_NC[flags] = build_program(*flags)
    return _CACHED_NC[flags]


def _make_consts():
    hh = np.arange(H)
    pp = np.arange(P)
    cc = np.arange(DC)
    selsum = (hh[None, None, :] == 2 * cc[None, :, None] + pp[:, None, None] // 64)
    selbc = (hh[:, None, None] == 2 * cc[None, :, None] + pp[None, None, :] // 64)
    return {
        "ident": np.eye(P, dtype=np.float32),
        "selsum": selsum.astype(NPBF16),
        "selbc": selbc.astype(NPBF16),
        "ones_row": np.ones((1, P), dtype=NPBF16),
    }


def make_in_maps(x, w_q, w_k, w_v, w_o, w_ff1, b_ff1, w_ff2, b_ff2, g1, b1, g2, b2):
    f = np.float32
    shared = {
        "wq": np.asarray(w_q, f).astype(NPBF16), "wk": np.asarray(w_k, f).astype(NPBF16),
        "wv": np.asarray(w_v, f).astype(NPBF16), "wo": np.asarray(w_o, f).astype(NPBF16),
        "wf1": np.asarray(w_ff1, f).astype(NPBF16), "wf2": np.asarray(w_ff2, f).astype(NPBF16),
        "bf1": np.ascontiguousarray(np.asarray(b_ff1, f).reshape(FC, P).T),
        "bf2": np.asarray(b_ff2, f).reshape(1, D).astype(NPBF16),
        "g1b": np.broadcast_to(np.asarray(g1, f), (P, D)).copy(),
        "b1b": np.broadcast_to(np.asarray(b1, f), (P, D)).copy(),
        "g2b": np.broadcast_to(np.asarray(g2, f), (P, D)).copy(),
        "b2b": np.broadcast_to(np.asarray(b2, f), (P, D)).copy(),
        **_make_consts(),
    }
    x = np.ascontiguousarray(np.asarray(x, f))
    return [{"x": x[ts(c, BPC)].reshape(T, D),
             "xb": x[ts(c, BPC)].reshape(T, D).astype(NPBF16),
             **shared} for c in range(NCORES)]


def _flags_for(inputs):
    f = np.float32
    gb1 = (np.array_equal(np.asarray(inputs["g1"], f), np.ones(D, f))
           and np.array_equal(np.asarray(inputs["b1"], f), np.zeros(D, f)))
    gb2 = (np.array_equal(np.asarray(inputs["g2"], f), np.ones(D, f))
           and np.array_equal(np.asarray(inputs["b2"], f), np.zeros(D, f)))
    bf2 = bool(np.any(np.asarray(inputs["b_ff2"], f)))
    bf1 = bool(np.any(np.asarray(inputs["b_ff1"], f)))
    return (not gb1, not gb2, bf2, bf1)


def run(in_maps, flags=(True, True, True, True), **kw):
    nc = _get_nc(flags)
    return run_bass_kernel_spmd(nc, in_maps, core_ids=list(range(NCORES)), **kw)


def kernel(**inputs):
    flags = _flags_for(inputs)
    res = run(make_in_maps(**inputs), flags=flags)
    out = np.concatenate([r["out"].reshape(BPC, S, D) for r in res.results], axis=0)
    return out.astype(np.float32)


# revision 11
# speedup vs baseline: 1.0244x; 1.0244x over previous
"""CosFormer layer kernel v2 for 8x Trainium2 (Bass/Tile), data-parallel over batch.

Restructured from baseline (454us) based on trace evidence:
- PE issues N=512 MMs at 216ns warm/unstalled (LDWEIGHTS fully hidden by the
  64-deep reorder window) -> the win is PE density, not LDW dedup.
- Baseline lost ~226us to HAM re-throttle (K=4/8) during attention stalls and
  ~72us to ACT table swaps (Exp<->Reciprocal<->Sqrt interleaving).
Changes:
- Whole layer software-pipelined at c-block granularity: batch b's attention
  interleaves with batch b+1's QKV proj/norms, batch b-1's Oproj/LN1 tail, and
  per-token-block FFN chunks -> dense PE stream end to end.
- ACT runs only exp/relu/copy (one table) during the main stream; norms use raw
  Rsqrt batched per batch; softmax denominator reciprocal moved to DVE
  (reciprocal_approx_fast); LN2 Rsqrt fuses +eps via the bias operand.
- h1 -> h1T transposes moved from PE (64 transpose MMs + 64 ACT copies) to DMA
  xbar transposes ([128,128] bf16 blocks) on otherwise-idle DMA queues.
"""

import sys

if "/opt/trn_rl_repo" not in sys.path:
    sys.path.insert(0, "/opt/trn_rl_repo")

import ml_dtypes
import numpy as np

import concourse.bass as bass
import concourse.tile as tile
from concourse import mybir
from concourse.bass_utils import run_bass_kernel_spmd

F32 = mybir.dt.float32
BF16 = mybir.dt.bfloat16
NPBF16 = ml_dtypes.bfloat16
AX = mybir.AxisListType
AF = mybir.ActivationFunctionType
OP = mybir.AluOpType

# problem constants
B, S, D = 32, 512, 512
H, DK, DV, DFF = 8, 64, 64, 2048
TEMP = float(np.sqrt(DK))
LN_EPS = 1e-5
NCORES = 8
BPC = B // NCORES          # batches per core
T = BPC * S                # tokens per core
DC = D // 128              # d chunks
FC = DFF // 128            # dff chunks
SB = S // 128              # token chunks per batch
P = 128

# h1 -> h1T via DMA xbar transpose (frees 64 PE transposes + 64 ACT copies).
# Flip to False to fall back to PE transposes if the SBUF->SBUF path misbehaves.
USE_DMA_TRANSPOSE = True


def ts(i, n):
    return slice(i * n, (i + 1) * n)


# walrus codegen caps on semaphore-wait commands per instruction (empirical);
# excess waits are moved onto chained same-engine NOPs ahead of the instruction.
_WAIT_CAPS = {}
_DEFAULT_WAIT_CAP = 1
_NOP_WAIT_CAP = 1


def _legalize_waits(nc):
    nop_id = [0]
    for f in nc.m.functions:
        for bb in f.blocks:
            insts = bb.instructions
            i = 0
            while i < len(insts):
                ins = insts[i]
                si = ins.sync_info
                cap = _WAIT_CAPS.get(type(ins).__name__, _DEFAULT_WAIT_CAP)
                if si is not None and si.on_wait and len(si.on_wait) > cap:
                    waits = list(si.on_wait)
                    keep = waits[-cap:] if cap > 0 else []
                    excess = waits[: len(waits) - cap]
                    new_nops = []
                    for j in range(0, len(excess), _NOP_WAIT_CAP):
                        chunk = excess[j: j + _NOP_WAIT_CAP]
                        nop = mybir.InstNoOp(
                            name=f"waitnop-{nop_id[0]}",
                            engine=ins.engine,
                            ins=[],
                            outs=[],
                            sync_info=mybir.SyncInfo(on_wait=chunk, on_update=[]),
                        )
                        nop_id[0] += 1
                        nc.register_instruction(nop)
                        new_nops.append(nop)
                    si.on_wait[:] = keep
                    insts[i:i] = new_nops
                    i += len(new_nops)
                i += 1


def _dedup_ldweights(nc):
    """Remove an InstLdweights when the PE array already holds the same
    weights (sync-safe; waits migrate onto the next kept PE instruction)."""
    for f in nc.m.functions:
        for bb in f.blocks:
            insts = bb.instructions
            out = []
            last_w = None
            pend_waits = []
            for ins in insts:
                tn = type(ins).__name__
                if tn == "InstLdweights":
                    w = str(ins.ins[0])
                    if w == last_w:
                        si = ins.sync_info
                        if si is not None and si.on_wait:
                            pend_waits.extend(list(si.on_wait))
                        continue
                    last_w = w
                elif tn == "InstMatmult":
                    if getattr(ins, "is_transpose", False):
                        last_w = None
                if pend_waits and tn in ("InstLdweights", "InstMatmult"):
                    si = ins.sync_info
                    if si is None:
                        ins.sync_info = mybir.SyncInfo(on_wait=list(pend_waits),
                                                       on_update=[])
                    else:
                        si.on_wait.extend(pend_waits)
                    pend_waits = []
                out.append(ins)
            assert not pend_waits
            insts[:] = out


def _act_raw(nc, out, in_, func, scale=1.0, bias_ap=None):
    """Raw ACT-engine activation bypassing bass's accuracy guard (used for
    Reciprocal / Rsqrt; measured err ~1e-5 rel - far below bf16 noise)."""
    eng = nc.scalar
    inputs = [eng.lower_ap(in_)]
    if bias_ap is not None:
        inputs.append(eng.lower_ap(bias_ap))
    else:
        inputs.append(mybir.ImmediateValue(dtype=mybir.dt.float32, value=0.0))
    inputs.append(mybir.ImmediateValue(dtype=mybir.dt.float32, value=scale))
    inputs.append(mybir.ImmediateValue(dtype=mybir.dt.float32, value=0.0))
    return eng.add_instruction(
        mybir.InstActivation(
            name=nc.get_next_instruction_name(),
            func=func,
            ins=inputs,
            outs=[eng.lower_ap(out)],
        )
    )


def _act_rsqrt(nc, out, in_, scale=1.0, bias_ap=None):
    """out = 1/sqrt(in_*scale + bias). bias is an AP (walrus wants table-based
    activations' bias as an AP); defaults to the const-0 AP."""
    if bias_ap is None:
        bias_ap = nc.const_aps.scalar_like(0.0, in_)
    return _act_raw(nc, out, in_, AF.Rsqrt, scale=scale, bias_ap=bias_ap)


def _fast_recip(nc, pool, x_ap, shape, out_dtype, tagbase):
    """y = 1/x via ACT Reciprocal (correctness-only paths)."""
    y = pool.tile(shape, out_dtype, tag=tagbase + "_y", name=tagbase + "_y")
    _act_raw(nc, y[:], x_ap, AF.Reciprocal)
    return y


def build_program(apply_gb1=True, apply_gb2=True, apply_bf2=True, apply_bf1=True):
    # eps guard: with var ~1 the eps=1e-5 shift perturbs rstd by ~5e-6 relative
    # (below bf16 noise) and LN2 renormalizes exactly; LN1 fast path (g1=1,b1=0,
    # b_ff1=0) only needs the mean-subtract.
    ln1_fast = (not apply_gb1) and (not apply_bf1)
    nc = bass.Bass("TRN2", target_bir_lowering=False, debug=False)

    # ---- DRAM I/O ----
    x_d = nc.dram_tensor("x", [T, D], F32, kind="ExternalInput")
    xb_d = nc.dram_tensor("xb", [T, D], BF16, kind="ExternalInput")
    wq_d = nc.dram_tensor("wq", [D, D], BF16, kind="ExternalInput")
    wk_d = nc.dram_tensor("wk", [D, D], BF16, kind="ExternalInput")
    wv_d = nc.dram_tensor("wv", [D, D], BF16, kind="ExternalInput")
    wo_d = nc.dram_tensor("wo", [D, D], BF16, kind="ExternalInput")
    wf1_d = nc.dram_tensor("wf1", [D, DFF], BF16, kind="ExternalInput")
    wf2_d = nc.dram_tensor("wf2", [DFF, D], BF16, kind="ExternalInput")
    bf1_d = nc.dram_tensor("bf1", [P, FC], F32, kind="ExternalInput")
    bf2_d = nc.dram_tensor("bf2", [1, D], BF16, kind="ExternalInput")
    g1_d = nc.dram_tensor("g1b", [P, D], F32, kind="ExternalInput")
    b1_d = nc.dram_tensor("b1b", [P, D], F32, kind="ExternalInput")
    g2_d = nc.dram_tensor("g2b", [P, D], F32, kind="ExternalInput")
    b2_d = nc.dram_tensor("b2b", [P, D], F32, kind="ExternalInput")
    id_d = nc.dram_tensor("ident", [P, P], F32, kind="ExternalInput")
    ssum_d = nc.dram_tensor("selsum", [P, DC, H], BF16, kind="ExternalInput")
    sbc_d = nc.dram_tensor("selbc", [H, DC, P], BF16, kind="ExternalInput")
    ones_d = nc.dram_tensor("ones_row", [1, P], BF16, kind="ExternalInput")
    out_d = nc.dram_tensor("out", [T, D], F32, kind="ExternalOutput")
    h1scr_d = nc.dram_tensor("h1scr", [T, D], BF16, kind="Internal")

    with tile.TileContext(nc) as tc:
        with tc.tile_pool(name="consts", bufs=1) as consts, \
             tc.tile_pool(name="pers", bufs=1) as pers, \
             tc.tile_pool(name="work", bufs=1) as wk_, \
             tc.tile_pool(name="batch", bufs=2) as bp, \
             tc.tile_pool(name="epool", bufs=3) as ep, \
             tc.tile_pool(name="tmp", bufs=2) as bt, \
             tc.tile_pool(name="psSC", bufs=2, space="PSUM") as psSC, \
             tc.tile_pool(name="psAV", bufs=1, space="PSUM") as psAV, \
             tc.tile_pool(name="psM", bufs=2, space="PSUM") as psM:

            # ---- persistent tensors + startup DMAs ----
            # Both HWDGE queues serialize descriptor generation (~0.7us per
            # DMA, ~1.3us per xbar transpose), so startup order = need order:
            # wq/wk + the hseg-0 xT transposes gate proj(0). Split the eight
            # xT transposes across sync and ACT queues; consts and the 8MB of
            # FFN weights come after.
            h1T = pers.tile([P, DC, T], BF16)
            # bf16 h1 (residual source + DMA-transpose source): the ~4e-3
            # rounding is renormalized by LN2; saves 16KB/partition SBUF.
            h1tok = pers.tile([P, T // P, D], BF16)
            xT = wk_.tile([P, DC, T], BF16)
            wq = pers.tile([P, DC, D], BF16)
            nc.sync.dma_start(wq[:], wq_d.ap().rearrange("(c p) n -> p c n", p=P))
            wk = pers.tile([P, DC, D], BF16)
            nc.sync.dma_start(wk[:], wk_d.ap().rearrange("(c p) n -> p c n", p=P))

            def xt_chunk(b):
                # per-batch [S,128]->[128,S] transposes, split across both
                # HWDGE queues so batch 0's chunks land first (~8us) and
                # proj(0) starts ~15us earlier than with half-T transposes
                for eng, clist in ((nc.sync, (0, 1)), (nc.scalar, (2, 3))):
                    for c in clist:
                        eng.dma_start_transpose(xT[:, c, ts(b, S)],
                                                xb_d[ts(b, S), ts(c, P)])
            xt_chunk(0)

            # ---- constants (ACT queue: sync carries the weight loads) ----
            ident = consts.tile([P, P], F32)
            nc.scalar.dma_start(ident[:], id_d[:])
            identb = None
            if not USE_DMA_TRANSPOSE:
                identb = consts.tile([P, P], BF16)
                nc.vector.tensor_copy(identb[:], ident[:])
            selsum = consts.tile([P, DC, H], BF16)
            nc.scalar.dma_start(selsum[:], ssum_d[:])
            selbc = consts.tile([H, DC, P], BF16)
            nc.scalar.dma_start(selbc[:], sbc_d[:])
            ones_row = consts.tile([1, P], BF16)
            nc.scalar.dma_start(ones_row[:], ones_d[:])
            g1b = b1b = g2b = b2b = bf1 = bf2 = None
            if apply_gb1:
                g1b = consts.tile([P, D], F32)
                nc.scalar.dma_start(g1b[:], g1_d[:])
                b1b = consts.tile([P, D], F32)
                nc.scalar.dma_start(b1b[:], b1_d[:])
            if apply_gb2:
                g2b = consts.tile([P, D], F32)
                nc.scalar.dma_start(g2b[:], g2_d[:])
                b2b = consts.tile([P, D], F32)
                nc.scalar.dma_start(b2b[:], b2_d[:])
            if apply_bf1:
                bf1 = consts.tile([P, FC], F32)
                nc.scalar.dma_start(bf1[:], bf1_d[:])
            if apply_bf2:
                bf2 = consts.tile([1, D], BF16)
                nc.scalar.dma_start(bf2[:], bf2_d[:])
            eps128 = consts.tile([P, 1], F32)
            nc.vector.memset(eps128[:], LN_EPS)

            wv = pers.tile([P, DC, D], BF16)
            nc.sync.dma_start(wv[:], wv_d.ap().rearrange("(c p) n -> p c n", p=P))
            wo = pers.tile([P, DC, D], BF16)
            nc.sync.dma_start(wo[:], wo_d.ap().rearrange("(c p) n -> p c n", p=P))
            xt_chunk(1)
            xt_chunk(2)
            xt_chunk(3)
            wf1 = pers.tile([P, DC, DFF], BF16)
            nc.sync.dma_start(wf1[:], wf1_d.ap().rearrange("(c p) n -> p c n", p=P))
            wf2 = pers.tile([P, FC, D], BF16)
            nc.sync.dma_start(wf2[:], wf2_d.ap().rearrange("(c p) n -> p c n", p=P))

            # per-batch state (cycled, bufs=2)
            QTs, KTs, Vbs, attbs, den8s, rden8s = {}, {}, {}, {}, {}, {}
            r8s = {}
            ffas = {}
            ln2st = {}

            # ---------------- emission helpers ----------------

            def run_gen(g):
                for _ in g:
                    pass

            def weave(a, b):
                """Alternate one chunk from each stream until both end."""
                a, b = iter(a), iter(b)
                alive = True
                while alive:
                    alive = False
                    for it in (a, b):
                        try:
                            next(it)
                            alive = True
                            yield
                        except StopIteration:
                            pass

            def chain(*gens):
                for g in gens:
                    for _ in g:
                        yield

            def proj_qk(b):
                """Q/K projections for batch b (feature-major, bf16).
                bufs=3: QT(b+2) is written (proj filler in S(b)) while
                scores(b, c>=2) still read QT(b)."""
                bcols = ts(b, S)
                QT = bp.tile([P, DC, S], BF16, tag="QT", name="QT", bufs=3)
                KT = bp.tile([P, DC, S], BF16, tag="KT", name="KT", bufs=3)
                QTs[b], KTs[b] = QT, KT
                for w_sb, XTt in ((wq, QT), (wk, KT)):
                    for cpair in range(DC // 2):
                        pp = psSC.tile([P, 2, S], F32, tag="pscr", name="pp")
                        for ci in range(2):
                            c = 2 * cpair + ci
                            for kc in range(DC):
                                nc.tensor.matmul(pp[:, ci, :],
                                                 w_sb[:, kc, ts(c, P)],
                                                 xT[:, kc, bcols],
                                                 start=(kc == 0),
                                                 stop=(kc == DC - 1))
                        for ci in range(2):
                            c = 2 * cpair + ci
                            nc.vector.tensor_copy(XTt[:, c, :], pp[:, ci, :])
                        yield

            def proj_v(b):
                """V projection for batch b (token-major, into augmented Vb).
                bufs=3: proj_v(b+2) may be pulled while AV(b) still reads
                Vb(b)."""
                Vb = bp.tile([P, SB, H, DV + 1], BF16, tag="Vb", name="Vb",
                             bufs=3)
                Vbs[b] = Vb
                nc.gpsimd.memset(Vb[:, :, :, DV:DV + 1], 1.0)
                for qpair in range(SB // 2):
                    pv = psSC.tile([P, 2, S], F32, tag="pscr", name="pv")
                    for i in range(2):
                        q = 2 * qpair + i
                        for kc in range(DC):
                            nc.tensor.matmul(pv[:, i, :],
                                             xT[:, kc, ts(b * SB + q, P)],
                                             wv[:, kc, :],
                                             start=(kc == 0), stop=(kc == DC - 1))
                    for i in range(2):
                        q = 2 * qpair + i
                        nc.vector.tensor_copy(
                            Vb[:, q, :, 0:DV],
                            pv[:, i, :].rearrange("p (h d) -> p h d", h=H))
                    yield

            def norms(b):
                """rq=1/(temp*||q||), rk=1/||k|| via Square + selector-matmul
                column sums + one raw Rsqrt per side."""
                for w2, (XTt, scale) in enumerate(
                        ((QTs[b], TEMP * TEMP), (KTs[b], 1.0))):
                    ps8 = psM.tile([H, S], F32, tag="misc", name="ps8")
                    for c in range(DC):
                        sq = bt.tile([P, S], BF16, tag="sq", name="sq")
                        # Square on ACT (same table as Exp) to offload DVE
                        nc.scalar.activation(sq[:], XTt[:, c, :], AF.Square)
                        nc.tensor.matmul(ps8[:], selsum[:, c, :], sq[:],
                                         start=(c == 0), stop=(c == DC - 1))
                    r8 = bt.tile([H, S], BF16, tag=f"r8_{w2}", name="r8",
                                 bufs=1)
                    _act_rsqrt(nc, r8[:], ps8[:], scale=scale)
                    r8s[(b, w2)] = r8
                    yield

            def folds(b, half):
                """Fold rq/rk into QT/KT via matmul-broadcast [8,S]->[128,S].
                half 0 -> c 0..1, half 1 -> c 2..3 (split for interleaving)."""
                for c in range(2 * half, 2 * half + 2):
                    for w2, XTt in enumerate((QTs[b], KTs[b])):
                        pb = psM.tile([P, S], F32, tag="misc", name="pb")
                        nc.tensor.matmul(pb[:], selbc[:, c, :], r8s[(b, w2)][:],
                                         start=True, stop=True)
                        nc.vector.tensor_mul(XTt[:, c, :], XTt[:, c, :], pb[:])
                    yield

            def attn_alloc(b):
                attbs[b] = bp.tile([P, DC, S], BF16, tag="attb", name="attb")
                den8s[b] = bt.tile([H, S], F32, tag="den8", name="den8")

            def attn_part1(b, c):
                """scores + exp + AV accumulation for head pair c of batch b.
                Scores run one j ahead of the AVs so the exp latency is
                covered; a filler chunk is pulled at every yield. Evacuates
                attb UNSCALED and DMA-packs the denominator rows into den8."""
                QT, KT, Vb = QTs[b], KTs[b], Vbs[b]
                attb = attbs[b]
                pav0 = psAV.tile([DV + 1, S], F32, tag="pav0", name="pav0")
                pav1 = psAV.tile([DV + 1, S], F32, tag="pav1", name="pav1")

                def scores(j):
                    pscr = psSC.tile([P, 2, S], F32, tag="pscr", name="pscr")
                    for h in range(2):
                        r0 = 64 * h
                        nc.tensor.matmul(pscr[:, h, :],
                                         KT[r0:r0 + 64, c, ts(j, P)],
                                         QT[r0:r0 + 64, c, :],
                                         start=True, stop=True)
                    e = ep.tile([P, 2, S], BF16, tag="e", name="e")
                    nc.scalar.activation(e[:], pscr[:], AF.Exp)
                    return e

                es = [scores(0)]
                for j in range(SB):
                    if j + 1 < SB:
                        es.append(scores(j + 1))
                    yield
                    for h, pav in ((0, pav0), (1, pav1)):
                        nc.tensor.matmul(pav[:], Vb[:, j, 2 * c + h, :],
                                         es[j][:, h, :],
                                         start=(j == 0), stop=(j == SB - 1))
                    yield
                for h, pav in ((0, pav0), (1, pav1)):
                    nc.vector.tensor_copy(attb[64 * h:64 * h + 64, c, :],
                                          pav[0:DV, :])
                    # den row evac on ACT (Copy shares the Exp table) then
                    # DMA SBUF->SBUF to pack partition row 2c+h of den8
                    den_sb = bt.tile([1, S], F32, tag="densb", name="densb")
                    nc.scalar.copy(den_sb[:], pav[DV:DV + 1, :])
                    r = 2 * c + h
                    nc.sync.dma_start(den8s[b][r:r + 1, :], den_sb[:])
                yield

            def attn_finish_a(b):
                """One [8,S] reciprocal for the batch (DVE only)."""
                rden8 = bt.tile([H, S], F32, tag="rden8", name="rden8", bufs=1)
                nc.vector.reciprocal(rden8[:], den8s[b][:])
                rden8b = bt.tile([H, S], BF16, tag="rden8b", name="rden8b",
                                 bufs=1)
                nc.vector.tensor_copy(rden8b[:], rden8[:])
                rden8s[b] = rden8b
                yield

            def attn_finish_b(b):
                """Per-c broadcast (selbc matmul) + in-place scale of attb."""
                attb = attbs[b]
                rden8b = rden8s.pop(b)
                for c in range(DC):
                    pb = psM.tile([P, S], F32, tag="misc", name="pbden")
                    nc.tensor.matmul(pb[:], selbc[:, c, :], rden8b[:],
                                     start=True, stop=True)
                    nc.vector.tensor_mul(attb[:, c, :], attb[:, c, :], pb[:])
                    yield

            def attn_finish(b):
                for _ in attn_finish_a(b):
                    yield
                for _ in attn_finish_b(b):
                    yield

            def tail_xt2(b):
                """Prefetch this batch's residual x rows (token-major, f32)."""
                tiles = []
                for q in range(SB):
                    xt2 = bt.tile([P, D], F32, tag=f"xt2_{q}", name="xt2",
                                  bufs=1)
                    nc.sync.dma_start(xt2[:], x_d[ts(b * SB + q, P), :])
                    tiles.append(xt2)
                return tiles

            def tail_q(b, q, xt2):
                """O-projection + residual + LN1 + h1 scratch store."""
                attb = attbs[b]
                po = psM.tile([P, D], F32, tag="misc", name="po")
                for c in range(DC):
                    nc.tensor.matmul(po[:], attb[:, c, ts(q, P)], wo[:, c, :],
                                     start=(c == 0), stop=(c == DC - 1))
                idx = b * SB + q
                h1 = h1tok[:, idx, :]
                if ln1_fast:
                    r1 = bt.tile([P, D], F32, tag="r1", name="r1", bufs=1)
                    s1 = bt.tile([P, 1], F32, tag="s1", name="s1")
                    nc.vector.scalar_tensor_tensor(
                        r1[:], po[:], 1.0, xt2[:],
                        op0=OP.mult, op1=OP.add, accum_out=s1[:])
                    nm = bt.tile([P, 1], F32, tag="nm", name="nm")
                    nc.vector.tensor_scalar_mul(nm[:], s1[:], -1.0 / D)
                    nc.vector.tensor_scalar_add(h1, r1[:], nm[:])
                else:
                    r1 = bt.tile([P, D], F32, tag="r1", name="r1", bufs=1)
                    nc.vector.tensor_add(r1[:], po[:], xt2[:])
                    bst = bt.tile([P, 6], F32, tag="bst", name="bst")
                    nc.vector.bn_stats(bst[:], r1[:])
                    mv = bt.tile([P, 2], F32, tag="mv", name="mv")
                    nc.vector.bn_aggr(mv[:], bst[:])
                    rstd = bt.tile([P, 1], F32, tag="rstd", name="rstd")
                    _act_rsqrt(nc, rstd[:], mv[:, 1:2], bias_ap=eps128[:])
                    nc.vector.tensor_scalar(h1, r1[:], mv[:, 0:1], rstd[:],
                                            OP.subtract, OP.mult)
                    if apply_gb1:
                        nc.vector.tensor_mul(h1, h1, g1b[:])
                        nc.vector.tensor_add(h1, h1, b1b[:])
                if USE_DMA_TRANSPOSE:
                    # store token-major h1 to scratch DRAM; load_h1T() later
                    # transpose-loads a whole batch in 4 big xbar transposes
                    nc.sync.dma_start(h1scr_d[ts(idx, P), :], h1tok[:, idx, :])
                else:
                    for c in range(DC):
                        pt2 = psM.tile([P, P], BF16, tag="miscT", name="pt2")
                        nc.tensor.transpose(pt2[:], h1tok[:, idx, ts(c, P)],
                                            identb[:])
                        nc.vector.tensor_copy(h1T[:, c, ts(idx, P)], pt2[:])
                yield

            def load_h1T(b):
                """Transpose-load batch b's h1 (DRAM scratch, token-major)
                into feature-major h1T: 4 [S,128]->[128,S] xbar transposes."""
                if not USE_DMA_TRANSPOSE:
                    return
                for c in range(DC):
                    nc.sync.dma_start_transpose(
                        h1T[:, c, ts(b, S)], h1scr_d[ts(b, S), ts(c, P)])

            def ffn1_chunk(tb, f_lo, f_hi):
                """FFN1 (relu(W1^T h1^T)) for f chunks [f_lo, f_hi).
                ffa bufs=1: the schedule finishes FFN2(tb) before FFN1(tb+1)
                starts, so one slot cycles."""
                if tb not in ffas:
                    ffas[tb] = bp.tile([P, FC, S], BF16, tag="ffa",
                                       name="ffa", bufs=1)
                ffa = ffas[tb]
                for f in range(f_lo, f_hi):
                    pf = psM.tile([P, S], F32, tag="misc", name="pf")
                    for c in range(DC):
                        nc.tensor.matmul(pf[:], wf1[:, c, ts(f, P)],
                                         h1T[:, c, ts(tb, S)],
                                         start=(c == 0), stop=(c == DC - 1))
                    relu_bias = bf1[:, f:f + 1] if apply_bf1 else 0.0
                    nc.scalar.activation(ffa[:, f, :], pf[:],
                                         AF.Relu, bias=relu_bias)
                    yield

            def ffn2_q(tb, q):
                """FFN2 + residual + LN2 stats for token block q of tb."""
                if tb not in ln2st:
                    var4 = bt.tile([P, SB], F32, tag="var4", name="var4")
                    mean4 = bt.tile([P, SB], F32, tag="mean4", name="mean4")
                    ln2st[tb] = (var4, mean4, [])
                var4, mean4, r2s = ln2st[tb]
                ffa = ffas[tb]
                p2 = psM.tile([P, D], F32, tag="misc", name="p2")
                for f in range(FC):
                    nc.tensor.matmul(p2[:], ffa[:, f, ts(q, P)],
                                     wf2[:, f, :], start=(f == 0),
                                     stop=(not apply_bf2 and f == FC - 1))
                    if f % 4 == 3 and f != FC - 1:
                        yield
                if apply_bf2:
                    nc.tensor.matmul(p2[:], ones_row[:], bf2[:],
                                     start=False, stop=True)
                r2 = bt.tile([P, D], F32, tag=f"r2_{q}", name="r2", bufs=1)
                nc.vector.tensor_add(r2[:], p2[:], h1tok[:, tb * SB + q, :])
                bst2 = bt.tile([P, 6], F32, tag="bst2", name="bst2")
                nc.vector.bn_stats(bst2[:], r2[:])
                mv2 = bt.tile([P, 2], F32, tag="mv2", name="mv2")
                nc.vector.bn_aggr(mv2[:], bst2[:])
                nc.vector.tensor_copy(var4[:, q:q + 1], mv2[:, 1:2])
                nc.vector.tensor_copy(mean4[:, q:q + 1], mv2[:, 0:1])
                r2s.append(r2)
                yield

            def ln2_finish(tb):
                var4, mean4, r2s = ln2st.pop(tb)
                rstd4 = bt.tile([P, SB], F32, tag="rstd4", name="rstd4")
                _act_rsqrt(nc, rstd4[:], var4[:], bias_ap=eps128[:])
                for q in range(SB):
                    # normalize in place (r2 tiles are per-q): the 4 q-chains
                    # stay independent so the out-DMAs pipeline
                    y = r2s[q]
                    nc.vector.tensor_scalar(y[:], y[:], mean4[:, q:q + 1],
                                            rstd4[:, q:q + 1],
                                            OP.subtract, OP.mult)
                    if apply_gb2:
                        nc.vector.tensor_mul(y[:], y[:], g2b[:])
                        nc.vector.tensor_add(y[:], y[:], b2b[:])
                    nc.sync.dma_start(out_d[ts(tb * SB + q, P), :], y[:])

            # ---------------- pipelined schedule ----------------
            # pre: batches 0,1 projected+normed, batch 0 folded
            for g in (proj_qk(0), proj_v(0), norms(0), proj_qk(1), proj_v(1),
                      folds(0, 0), folds(0, 1), norms(1)):
                run_gen(g)

            def seg(b, stream):
                """A(b) c-blocks; one filler chunk pulled per part1 yield,
                remainder drained at segment end."""
                attn_alloc(b)
                it = iter(stream)
                for c in range(DC):
                    for _ in attn_part1(b, c):
                        next(it, None)
                for _ in it:
                    pass

            def tails(b, xt2s):
                return chain(*(tail_q(b, q, xt2s[q]) for q in range(SB)))

            # S0: A(0) + folds(1) + proj(2) + norms(2)
            # norms(b+2) last: its ACT Rsqrt (different table) lands after
            # this segment's Exps.
            seg(0, chain(folds(1, 0), folds(1, 1), proj_qk(2), proj_v(2),
                         norms(2)))

            xt2s0 = tail_xt2(0)
            # S1: A(1) + finish(0)/T(0) + folds(2) + proj(3) + norms(3)
            seg(1, chain(attn_finish(0),
                         weave(tails(0, xt2s0),
                               chain(folds(2, 0), proj_qk(3), folds(2, 1),
                                     proj_v(3))),
                         norms(3)))
            load_h1T(0)
            xt2s1 = tail_xt2(1)

            # S2: A(2) + finish(1)/T(1) + folds(3) + FFN1(0)
            # (ffa bufs=1 discipline: FFN2(tb) fully precedes FFN1(tb+1))
            seg(2, chain(attn_finish(1),
                         weave(tails(1, xt2s1),
                               chain(folds(3, 0), folds(3, 1),
                                     ffn1_chunk(0, 0, 16)))))
            load_h1T(1)
            xt2s2 = tail_xt2(2)

            # S3: A(3) + finish(2)/T(2) + FFN2(0) then FFN1(1)
            seg(3, chain(attn_finish(2),
                         weave(tails(2, xt2s2),
                               chain(ffn2_q(0, 0), ffn2_q(0, 1),
                                     ffn2_q(0, 2), ffn2_q(0, 3),
                                     ffn1_chunk(1, 0, 16)))))
            load_h1T(2)
            xt2s3 = tail_xt2(3)

            # S4: finish(3) + T(3) + FFN2(1), then FFN1(2)
            run_gen(attn_finish(3))
            ln2_finish(0)
            for q in range(SB):
                run_gen(tail_q(3, q, xt2s3[q]))
                run_gen(ffn2_q(1, q))
            load_h1T(3)
            run_gen(ffn1_chunk(2, 0, 8))
            ln2_finish(1)
            run_gen(ffn1_chunk(2, 8, 16))
            # S5: FFN2(2), then FFN1(3)
            for q in range(SB):
                run_gen(ffn2_q(2, q))
            run_gen(ffn1_chunk(3, 0, 8))
            ln2_finish(2)
            run_gen(ffn1_chunk(3, 8, 16))
            # S6: FFN2(3)
            for q in range(SB):
                run_gen(ffn2_q(3, q))
            ln2_finish(3)

    _dedup_ldweights(nc)
    _legalize_waits(nc)
    return nc


_CACHED_NC = {}


def _get_nc(flags):
    if flags not in _CACHED_NC:
        _CACHED_NC[flags] = build_program(*flags)
    return _CACHED_NC[flags]


def _make_consts():
    hh = np.arange(H)
    pp = np.arange(P)
    cc = np.arange(DC)
    selsum = (hh[None, None, :] == 2 * cc[None, :, None] + pp[:, None, None] // 64)
    selbc = (hh[:, None, None] == 2 * cc[None, :, None] + pp[None, None, :] // 64)
    return {
        "ident": np.eye(P, dtype=np.float32),
        "selsum": selsum.astype(NPBF16),
        "selbc": selbc.astype(NPBF16),
        "ones_row": np.ones((1, P), dtype=NPBF16),
    }


def make_in_maps(x, w_q, w_k, w_v, w_o, w_ff1, b_ff1, w_ff2, b_ff2, g1, b1, g2, b2):
    f = np.float32
    shared = {
        "wq": np.asarray(w_q, f).astype(NPBF16), "wk": np.asarray(w_k, f).astype(NPBF16),
        "wv": np.asarray(w_v, f).astype(NPBF16), "wo": np.asarray(w_o, f).astype(NPBF16),
        "wf1": np.asarray(w_ff1, f).astype(NPBF16), "wf2": np.asarray(w_ff2, f).astype(NPBF16),
        "bf1": np.ascontiguousarray(np.asarray(b_ff1, f).reshape(FC, P).T),
        "bf2": np.asarray(b_ff2, f).reshape(1, D).astype(NPBF16),
        "g1b": np.broadcast_to(np.asarray(g1, f), (P, D)).copy(),
        "b1b": np.broadcast_to(np.asarray(b1, f), (P, D)).copy(),
        "g2b": np.broadcast_to(np.asarray(g2, f), (P, D)).copy(),
        "b2b": np.broadcast_to(np.asarray(b2, f), (P, D)).copy(),
        **_make_consts(),
    }
    x = np.ascontiguousarray(np.asarray(x, f))
    return [{"x": x[ts(c, BPC)].reshape(T, D),
             "xb": x[ts(c, BPC)].reshape(T, D).astype(NPBF16),
             **shared} for c in range(NCORES)]


def _flags_for(inputs):
    f = np.float32
    gb1 = (np.array_equal(np.asarray(inputs["g1"], f), np.ones(D, f))
           and np.array_equal(np.asarray(inputs["b1"], f), np.zeros(D, f)))
    gb2 = (np.array_equal(np.asarray(inputs["g2"], f), np.ones(D, f))
           and np.array_equal(np.asarray(inputs["b2"], f), np.zeros(D, f)))
    bf2 = bool(np.any(np.asarray(inputs["b_ff2"], f)))
    bf1 = bool(np.any(np.asarray(inputs["b_ff1"], f)))
    return (not gb1, not gb2, bf2, bf1)


def run(in_maps, flags=(True, True, True, True), **kw):
    nc = _get_nc(flags)
    return run_bass_kernel_spmd(nc, in_maps, core_ids=list(range(NCORES)), **kw)


def kernel(**inputs):
    flags = _flags_for(inputs)
    res = run(make_in_maps(**inputs), flags=flags)
    out = np.concatenate([r["out"].reshape(BPC, S, D) for r in res.results], axis=0)
    return out.astype(np.float32)
